# revision 1
# baseline (speedup 1.0000x reference)
"""Trainium2 Bass kernel for 2-layer GAT (nn_GAT_66821101191576).

Self-contained: hardcodes shapes, does host-side graph preprocessing,
builds/compiles a Tile/Bass SPMD program for 8 NeuronCores, runs via
run_bass_kernel_spmd, and reassembles the full output.
"""

import contextlib
import ctypes
import sys
import types

for _p in ("/opt/trn_rl_repo", "/opt/pypackages"):
    if _p not in sys.path:
        sys.path.insert(0, _p)

import numpy as np

import concourse.bacc as bacc
import concourse.bass as bass
import concourse.mybir as mybir
import concourse.tile as tile
from concourse.bass_utils import run_bass_kernel_spmd

F32 = mybir.dt.float32
F32R = mybir.dt.float32r
BF16 = mybir.dt.bfloat16
I16 = mybir.dt.int16
AF = mybir.ActivationFunctionType
OP = mybir.AluOpType

# ---------------------------------------------------------------- problem dims
N = 20000
E = 640000
IN = 256
H = 8
HF = 64
OUT = 32
NEG_SLOPE = 0.2

NCORES = 8
NEG_BIG = -1.0e30

# last profiling result (ns), for test harnesses
LAST_EXEC_TIME_NS = None


def _install_ntff_hook():
    """Provide antenv.axon_hooks (NTFF profiling) if the image lacks it."""
    try:
        from antenv.axon_hooks import get_axon_ntff_profile_hook  # noqa: F401
        return
    except ImportError:
        pass
    so_path = "/opt/axon/libaxon_pjrt.so"
    try:
        lib = ctypes.CDLL(so_path)
    except OSError:
        return
    if not hasattr(lib, "axon_start_nrt_profile"):
        return
    lib.axon_start_nrt_profile.argtypes = [
        ctypes.POINTER(ctypes.c_int64),
        ctypes.c_size_t,
    ]
    lib.axon_start_nrt_profile.restype = ctypes.c_int64
    lib.axon_stop_nrt_profile.argtypes = [ctypes.c_char_p]
    lib.axon_stop_nrt_profile.restype = ctypes.c_int64

    @contextlib.contextmanager
    def _hook(output_dir, device_ids):
        import jax

        jax.devices()
        if device_ids:
            ids = (ctypes.c_int64 * len(device_ids))(*device_ids)
            rc = lib.axon_start_nrt_profile(ids, len(device_ids))
        else:
            rc = lib.axon_start_nrt_profile(None, 0)
        if rc != 0:
            raise RuntimeError(f"axon_start_nrt_profile rc={rc}")
        try:
            yield
        finally:
            n = lib.axon_stop_nrt_profile(str(output_dir).encode())
            print(f"ntff profile: {n} file(s) -> {output_dir}", file=sys.stderr)

    mod = types.ModuleType("antenv.axon_hooks")
    mod.get_axon_ntff_profile_hook = lambda: _hook
    mod.set_axon_ntff_profile_hook = lambda h: None
    sys.modules["antenv.axon_hooks"] = mod
    import antenv

    antenv.axon_hooks = mod


# ------------------------------------------------------------ host preprocessing
def _wrap_idx16(flat):
    """Wrap a flat int16 index vector into the dma_gather SBUF image:
    index i -> partition i%16, column i//16, replicated 8x down partitions."""
    assert flat.size % 16 == 0
    v = flat.reshape(-1, 16).T.astype(np.int16)  # [16, n/16]
    return np.tile(v, (8, 1))  # [128, n/16]


def preprocess(x, edge_index, W1, att_src1, att_dst1, b1, W2, att_src2, att_dst2, b2):
    """Build per-core inputs + compile-time metadata."""
    n = N
    src = np.asarray(edge_index[0], dtype=np.int64)
    dst = np.asarray(edge_index[1], dtype=np.int64)
    loops = np.arange(n, dtype=np.int64)
    src = np.concatenate([src, loops])
    dst = np.concatenate([dst, loops])

    deg = np.bincount(dst, minlength=n)
    order = np.argsort(-deg, kind="stable")  # rank -> node
    ranks = np.arange(n)
    # rank r -> core r%NCORES, local slot r//NCORES
    PN = ((n + NCORES - 1) // NCORES + 127) // 128 * 128  # padded nodes/core
    CH = PN // 128
    NSLOT = NCORES * PN
    PADROW = NSLOT
    NROWS = NSLOT + 128

    node2slot = np.empty(n, dtype=np.int64)
    node2slot[order] = (ranks % NCORES) * PN + ranks // NCORES

    es = node2slot[src]
    ed = node2slot[dst]

    o2 = np.argsort(ed, kind="stable")
    s_src = es[o2]
    s_dst = ed[o2]
    starts = np.searchsorted(s_dst, np.arange(NSLOT))
    kwithin = np.arange(s_dst.size) - starts[s_dst]

    deg_slot = np.zeros(NSLOT, dtype=np.int64)
    deg_slot[: node2slot.max() + 1] = np.bincount(ed, minlength=NSLOT)[: node2slot.max() + 1]
    deg_slot = np.bincount(ed, minlength=NSLOT)
    # per (local chunk c): max degree over all cores' chunk-c slots
    dmat = deg_slot.reshape(NCORES, CH, 128)
    dsched = dmat.max(axis=(0, 2))
    dsched = np.maximum(((dsched + 7) // 8) * 8, 8).astype(np.int64)  # mult of 8
    choff = np.concatenate([[0], np.cumsum(dsched)])
    TOTD = int(choff[-1])

    # fill per-core [TOTD, 128] tables with PADROW
    arr = np.full((NCORES, TOTD, 128), PADROW, dtype=np.int64)
    e_core = s_dst // PN
    e_loc = s_dst % PN
    e_ch = e_loc // 128
    e_p = e_loc % 128
    arr[e_core, choff[e_ch] + kwithin, e_p] = s_src
    assert arr.max() <= 32767

    idx_imgs = [_wrap_idx16(arr[k].reshape(-1).astype(np.int16)) for k in range(NCORES)]
    own_imgs = [
        _wrap_idx16(np.arange(k * PN, (k + 1) * PN, dtype=np.int16))
        for k in range(NCORES)
    ]

    # permuted node features, transposed, padded
    xT = np.zeros((IN, NROWS), dtype=np.float32)
    xT[:, node2slot] = np.asarray(x, dtype=np.float32).T

    # W extensions: columns reordered (c*H + h); attention projection folded in
    W1r = np.asarray(W1, np.float32).reshape(IN, H, HF)
    w1p = W1r.transpose(0, 2, 1).reshape(IN, H * HF)
    a1s = np.einsum("ihc,hc->ih", W1r, np.asarray(att_src1, np.float32))
    a1d = np.einsum("ihc,hc->ih", W1r, np.asarray(att_dst1, np.float32))
    w1e = np.concatenate([w1p, a1s, a1d], axis=1).astype(np.float32)  # [IN, 528]

    W2r = np.asarray(W2, np.float32).reshape(HF, H, OUT)
    w2p = W2r.transpose(0, 2, 1).reshape(HF, H * OUT)
    a2s = np.einsum("ihc,hc->ih", W2r, np.asarray(att_src2, np.float32))
    a2d = np.einsum("ihc,hc->ih", W2r, np.asarray(att_dst2, np.float32))
    w2e = np.concatenate([w2p, a2s, a2d], axis=1).astype(np.float32)  # [HF, 272]

    import ml_dtypes
    b1b = np.tile(np.asarray(b1, np.float32)[None, :], (128, 1))
    b2b = np.tile(np.asarray(b2, np.float32)[None, :], (128, 1))
    identf = np.eye(128, dtype=np.float32)
    identbf = np.eye(128, dtype=ml_dtypes.bfloat16)

    shared = {
        "xT": xT.astype(ml_dtypes.bfloat16),
        "w1e": w1e.astype(ml_dtypes.bfloat16),
        "w2e": w2e.astype(ml_dtypes.bfloat16),
        "b1b": b1b,
        "b2b": b2b,
        "identf": identf,
        "identbf": identbf,
    }
    in_maps = []
    for k in range(NCORES):
        m = dict(shared)
        m["idxs"] = idx_imgs[k]
        m["ownidx"] = own_imgs[k]
        in_maps.append(m)

    meta = {
        "PN": PN,
        "CH": CH,
        "NSLOT": NSLOT,
        "PADROW": PADROW,
        "NROWS": NROWS,
        "dsched": tuple(int(d) for d in dsched),
        "choff": tuple(int(c) for c in choff),
        "node2slot": node2slot,
    }
    return in_maps, meta


# ------------------------------------------------------------------ the program
def build_program(meta, ncores=NCORES, enable_asserts=False, debug=False):
    PN = meta["PN"]
    CH = meta["CH"]
    NSLOT = meta["NSLOT"]
    PADROW = meta["PADROW"]
    NROWS = meta["NROWS"]
    dsched = meta["dsched"]
    choff = meta["choff"]
    TOTD = choff[-1]
    NCH_ALL = NROWS // 128  # phase-A1 chunks (includes PADROW block)
    NCH_A2 = NSLOT // 128  # phase-A2 chunks
    D1 = H * HF  # 512
    D2 = H * OUT  # 256

    nc = bacc.Bacc(
        "TRN2",
        target_bir_lowering=False,
        debug=debug,
        enable_asserts=enable_asserts,
        num_devices=ncores,
        num_swdge_queues=4,
    )

    # ---- I/O
    xT = nc.dram_tensor("xT", [IN, NROWS], BF16, kind="ExternalInput")
    w1e = nc.dram_tensor("w1e", [IN, D1 + 16], BF16, kind="ExternalInput")
    w2e = nc.dram_tensor("w2e", [HF, D2 + 16], BF16, kind="ExternalInput")
    b1b_d = nc.dram_tensor("b1b", [128, HF], F32, kind="ExternalInput")
    b2b_d = nc.dram_tensor("b2b", [128, OUT], F32, kind="ExternalInput")
    identf_d = nc.dram_tensor("identf", [128, 128], F32, kind="ExternalInput")
    identbf_d = nc.dram_tensor("identbf", [128, 128], BF16, kind="ExternalInput")
    idxs_d = nc.dram_tensor("idxs", [128, TOTD * 8], I16, kind="ExternalInput")
    ownidx_d = nc.dram_tensor("ownidx", [128, PN // 16], I16, kind="ExternalInput")
    out_d = nc.dram_tensor("out", [PN, OUT], F32, kind="ExternalOutput")

    # ---- internal DRAM tables (rows: [features | asrc(8) adst(8) | pad])
    RW1 = D1 + 128  # 640 bf16 = 1280B
    RW2 = D2 + 128  # 384 bf16 = 768B
    h1_tbl = nc.dram_tensor("h1_tbl", [NROWS, RW1], BF16)
    h2_tbl = nc.dram_tensor("h2_tbl", [NROWS, RW2], BF16)
    h1t_loc = nc.dram_tensor("h1t_loc", [HF, PN], BF16)
    if ncores > 4:
        h1t_all = nc.dram_tensor("h1t_all", [ncores, HF, PN], BF16, addr_space="Shared")
    else:
        h1t_all = nc.dram_tensor("h1t_all", [ncores, HF, PN], BF16)

    _swctr = [0]
    _swprev = [None]

    def _gather(out_ap, in_ap, idxs_ap, nidx, elem):
        q = _swctr[0] % 4
        _swctr[0] += 1
        inst = nc.gpsimd.dma_gather(out_ap, in_ap, idxs_ap, nidx, nidx, elem, queue_num=q)
        if _swprev[0] is not None:
            bass._add_dep_helper(inst.ins, _swprev[0].ins, sync=False, reason="swdge order")
        _swprev[0] = inst
        return inst

    with tile.TileContext(nc) as tc:
        with contextlib.ExitStack() as big:
            cpool = big.enter_context(tc.tile_pool(name="consts", bufs=1))
            # constants
            w1_sb = cpool.tile([128, 2, D1 + 16], BF16)
            nc.sync.dma_start(
                w1_sb[:], w1e[:, :].rearrange("(k p) c -> p k c", p=128)
            )
            w2_sb = cpool.tile([HF, D2 + 16], BF16)
            nc.sync.dma_start(w2_sb[:], w2e[:, :])
            b1_sb = cpool.tile([128, HF], F32)
            nc.sync.dma_start(b1_sb[:], b1b_d[:, :])
            b2_sb = cpool.tile([128, OUT], F32)
            nc.sync.dma_start(b2_sb[:], b2b_d[:, :])
            idf_sb = cpool.tile([128, 128], F32)
            nc.sync.dma_start(idf_sb[:], identf_d[:, :])
            idb_sb = cpool.tile([128, 128], BF16)
            nc.sync.dma_start(idb_sb[:], identbf_d[:, :])
            idx_sb = cpool.tile([128, TOTD * 8], I16)
            nc.sync.dma_start(idx_sb[:], idxs_d[:, :])
            own_sb = cpool.tile([128, PN // 16], I16)
            nc.sync.dma_start(own_sb[:], ownidx_d[:, :])
            neg_sb = cpool.tile([1, 8], BF16)
            nc.vector.memset(neg_sb[:], NEG_BIG)
            zro_sb = cpool.tile([1, D2], BF16)
            nc.vector.memset(zro_sb[:], 0.0)
            eps_sb = cpool.tile([128, 1], F32)
            nc.vector.memset(eps_sb[:], 1e-30)

            # ---------------- phase A helper (projection into tables)
            def phase_A(nch, lhsT_of, w_sb, kparts, dcols, rw, h_tbl_, tag):
                fuse = dcols + 16 <= 512
                with contextlib.ExitStack() as st:
                    lp = st.enter_context(tc.tile_pool(name=f"a{tag}_l", bufs=4))
                    pp = st.enter_context(
                        tc.tile_pool(name=f"a{tag}_p", bufs=3, space="PSUM")
                    )
                    sp = st.enter_context(tc.tile_pool(name=f"a{tag}_s", bufs=3))
                    for g0 in range(0, nch, 8):
                        gn = min(8, nch - g0)
                        hst = sp.tile([128, 8, rw], BF16, tag=f"hst{tag}")
                        for r in range(gn):
                            g = g0 + r
                            lhsT = lhsT_of(lp, g)
                            if fuse:
                                ps = pp.tile([128, dcols + 16], F32, tag=f"ps{tag}")
                                for kk in range(kparts):
                                    nc.tensor.matmul(
                                        ps[:],
                                        lhsT(kk),
                                        w_sb(kk)[:, 0 : dcols + 16],
                                        start=(kk == 0),
                                        stop=(kk == kparts - 1),
                                    )
                                nc.scalar.copy(hst[:, r, 0 : dcols + 16], ps[:])
                            else:
                                ps = pp.tile([128, dcols], F32, tag=f"ps{tag}")
                                ps2 = pp.tile([128, 16], F32, tag=f"ps2{tag}")
                                for kk in range(kparts):
                                    nc.tensor.matmul(
                                        ps[:],
                                        lhsT(kk),
                                        w_sb(kk)[:, 0:dcols],
                                        start=(kk == 0),
                                        stop=(kk == kparts - 1),
                                    )
                                for kk in range(kparts):
                                    nc.tensor.matmul(
                                        ps2[:],
                                        lhsT(kk),
                                        w_sb(kk)[:, dcols : dcols + 16],
                                        start=(kk == 0),
                                        stop=(kk == kparts - 1),
                                    )
                                nc.scalar.copy(hst[:, r, 0:dcols], ps[:])
                                nc.vector.tensor_copy(
                                    hst[:, r, dcols : dcols + 16], ps2[:]
                                )
                        nc.sync.dma_start(
                            h_tbl_[g0 * 128 : (g0 + gn) * 128, 0 : dcols + 16].rearrange(
                                "(r p) c -> p r c", p=128
                            ),
                            hst[:, 0:gn, 0 : dcols + 16],
                        )

            # ---------------- phase A1
            def lhsT_of_A1(lp, g):
                xt = lp.tile([128, 2, 128], BF16, tag="xt")
                nc.sync.dma_start(
                    xt[:],
                    xT[:, g * 128 : (g + 1) * 128].rearrange(
                        "(k p) j -> p k j", p=128
                    ),
                )
                return lambda kk: xt[:, kk, :]

            phase_A(
                NCH_ALL,
                lhsT_of_A1,
                lambda kk: w1_sb[:, kk, :],
                2,
                D1,
                RW1,
                h1_tbl,
                "1",
            )
            # PADROW fixup: asrc(PADROW) = -inf
            nc.sync.dma_start(
                h1_tbl[PADROW : PADROW + 1, D1 : D1 + 8], neg_sb[:, 0:8]
            )

            # ---------------- gather/aggregate layer helper
            def phase_G(h_tbl_, dcols, rw, chans, b_sb, relu, sink, tag):
                """rows of h_tbl_: [features(dcols) | asrc(8) adst(8) | pad to rw].
                sink(c, sbuf_tile[128, chans]) consumes the chunk output."""
                with contextlib.ExitStack() as st:
                    gp = st.enter_context(tc.tile_pool(name=f"g{tag}_g", bufs=1))
                    hp = st.enter_context(tc.tile_pool(name=f"g{tag}_h", bufs=7))
                    sp = st.enter_context(tc.tile_pool(name=f"g{tag}_s", bufs=4))
                    mp = st.enter_context(tc.tile_pool(name=f"g{tag}_m", bufs=5))
                    pp = st.enter_context(
                        tc.tile_pool(name=f"g{tag}_p", bufs=4, space="PSUM")
                    )
                    # own-node rows (adst at cols dcols+8 : dcols+16)
                    atown = gp.tile([128, CH, rw], BF16, tag=f"atown{tag}")
                    off = 0
                    while off < PN:
                        nn = min(1024, PN - off)
                        _gather(
                            atown[:, off // 128 : (off + nn) // 128, :],
                            h_tbl_[:, :],
                            own_sb[:, off // 16 : (off + nn) // 16],
                            nn,
                            rw,
                        )
                        off += nn
                    for c in range(CH):
                        D = dsched[c]
                        col0 = choff[c] * 8
                        ex = mp.tile([128, D, H], BF16, tag=f"ex{tag}")
                        ps = pp.tile([128, dcols], F32, tag=f"ps{tag}")
                        adst_b = (
                            atown[:, c, dcols + 8 : dcols + 16]
                            .unsqueeze(1)
                            .broadcast_to([128, 8, H])
                        )
                        first = True
                        for d0 in range(0, D, 8):
                            hg = hp.tile([128, 8, rw], BF16, tag=f"hg{tag}")
                            _gather(
                                hg[:],
                                h_tbl_[:, :],
                                idx_sb[:, col0 + d0 * 8 : col0 + (d0 + 8) * 8],
                                1024,
                                rw,
                            )
                            # logits for this slice
                            epre = mp.tile([128, 8, H], F32, tag=f"epre{tag}")
                            nc.vector.tensor_tensor(
                                epre[:],
                                hg[:, :, dcols : dcols + 8],
                                adst_b,
                                OP.add,
                            )
                            e = mp.tile([128, 8, H], F32, tag=f"e{tag}")
                            nc.vector.scalar_tensor_tensor(
                                e[:], epre[:], NEG_SLOPE, epre[:], OP.mult, OP.max
                            )
                            nc.scalar.activation(ex[:, d0 : d0 + 8, :], e[:], AF.Exp)
                            sg = sp.tile([128, 8, dcols], BF16, tag=f"sg{tag}")
                            hg_v = hg[:, :, 0:dcols].rearrange(
                                "p d (c h) -> p d c h", h=H
                            )
                            sg_v = sg[:].rearrange("p d (c h) -> p d c h", h=H)
                            ex_v = (
                                ex[:, d0 : d0 + 8, :]
                                .unsqueeze(2)
                                .broadcast_to([128, 8, chans, H])
                            )
                            nc.vector.tensor_tensor(sg_v, hg_v, ex_v, OP.mult)
                            for j in range(8):
                                nc.tensor.matmul(
                                    ps[:],
                                    idb_sb[:],
                                    sg[:, j, :],
                                    start=first,
                                    stop=(d0 + 8 >= D and j == 7),
                                )
                                first = False
                        den = mp.tile([128, H], F32, tag=f"den{tag}")
                        nc.vector.reduce_sum(
                            den[:],
                            ex[:].transpose([0, 2, 1]),
                            axis=mybir.AxisListType.X,
                        )
                        nc.scalar.activation(den[:], den[:], AF.Identity, bias=eps_sb[:, 0:1])
                        rden = mp.tile([128, H], F32, tag=f"rden{tag}")
                        nc.vector.reciprocal(rden[:], den[:])
                        # evacuate PSUM early (frees bank for next chunk)
                        acc = mp.tile([128, dcols], F32, tag=f"acc{tag}")
                        nc.scalar.copy(acc[:], ps[:])
                        # normalize, mean over heads, bias
                        t1 = mp.tile([128, dcols], F32, tag=f"t1{tag}")
                        rden_b = rden[:].unsqueeze(1).broadcast_to([128, chans, H])
                        nc.vector.tensor_tensor(
                            t1[:].rearrange("p (c h) -> p c h", h=H),
                            acc[:].rearrange("p (c h) -> p c h", h=H),
                            rden_b,
                            OP.mult,
                        )
                        hsum = mp.tile([128, chans], F32, tag=f"hsum{tag}")
                        nc.vector.reduce_sum(
                            hsum[:],
                            t1[:].rearrange("p (c h) -> p c h", h=H),
                            axis=mybir.AxisListType.X,
                        )
                        res = mp.tile([128, chans], F32, tag=f"res{tag}")
                        nc.vector.scalar_tensor_tensor(
                            res[:], hsum[:], 1.0 / H, b_sb[:], OP.mult, OP.add
                        )
                        if relu:
                            resf = mp.tile([128, chans], F32, tag=f"resf{tag}")
                            nc.scalar.activation(resf[:], res[:], AF.Relu)
                        else:
                            resf = res
                        sink(c, resf)

            # ---------------- G1: layer-1 aggregation -> h1t_loc
            h1t_sb = cpool.tile([HF, PN], BF16)
            tp_pool = big.enter_context(
                tc.tile_pool(name="tpp", bufs=2, space="PSUM")
            )

            def sink1(c, resf):
                tps = tp_pool.tile([HF, 128], F32, tag="tps")
                nc.tensor.transpose(tps[:], resf[:], idf_sb[:])
                nc.scalar.copy(h1t_sb[:, c * 128 : (c + 1) * 128], tps[:])

            phase_G(h1_tbl, D1, RW1, HF, b1_sb, True, sink1, "1")
            nc.sync.dma_start(h1t_loc[:, :], h1t_sb[:])

            # ---------------- all-gather h1T
            if ncores > 1:
                nc.gpsimd.collective_compute(
                    "AllGather",
                    OP.bypass,
                    replica_groups=[list(range(ncores))],
                    ins=[h1t_loc[:, :]],
                    outs=[h1t_all[:, :, :]],
                )
            else:
                nc.sync.dma_start(h1t_all[0], h1t_loc[:, :])

            with tc.tile_pool(name="h1tp", bufs=1) as h1tp:
                h1tf = h1tp.tile([HF, NSLOT], BF16)
                nc.sync.dma_start(
                    h1tf[:].rearrange("p (j c) -> p j c", c=PN),
                    h1t_all[:, :, :].transpose([1, 0, 2]),
                )

                # ---------------- phase A2
                def lhsT_of_A2(lp, g):
                    return lambda kk: h1tf[:, g * 128 : (g + 1) * 128]

                phase_A(
                    NCH_A2,
                    lhsT_of_A2,
                    lambda kk: w2_sb,
                    1,
                    D2,
                    RW2,
                    h2_tbl,
                    "2",
                )
            nc.sync.dma_start(h2_tbl[PADROW : PADROW + 1, 0:D2], zro_sb[:, 0:D2])
            nc.sync.dma_start(
                h2_tbl[PADROW : PADROW + 1, D2 : D2 + 8], neg_sb[:, 0:8]
            )

            # ---------------- G2: layer-2 aggregation -> out
            op_pool = big.enter_context(tc.tile_pool(name="outp", bufs=2))
            ostage = [None]

            def sink2(c, resf):
                r = c % 4
                if r == 0:
                    ost = op_pool.tile([128, 4, OUT], F32, tag="ost")
                    ostage[0] = ost
                nc.vector.tensor_copy(ostage[0][:, r, :], resf[:])
                if r == 3 or c == CH - 1:
                    c0 = c - r
                    nc.sync.dma_start(
                        out_d[c0 * 128 : (c + 1) * 128, :].rearrange(
                            "(r p) c -> p r c", p=128
                        ),
                        ostage[0][:, 0 : r + 1, :],
                    )

            phase_G(h2_tbl, D2, RW2, OUT, b2_sb, False, sink2, "2")

    nc.compile()
    return nc


# ------------------------------------------------------------------ entry point
_CACHE = {}


def _get_program(meta):
    key = (meta["PN"], meta["dsched"])
    if key not in _CACHE:
        _CACHE[key] = build_program(meta)
    return _CACHE[key]


def kernel(x, edge_index, W1, att_src1, att_dst1, b1, W2, att_src2, att_dst2, b2,
           trace=False):
    global LAST_EXEC_TIME_NS
    _install_ntff_hook()
    in_maps, meta = preprocess(
        x, edge_index, W1, att_src1, att_dst1, b1, W2, att_src2, att_dst2, b2
    )
    nc = _get_program(meta)
    res = run_bass_kernel_spmd(
        nc, in_maps, list(range(NCORES)), trace=trace
    )
    LAST_EXEC_TIME_NS = res.exec_time_ns
    outs = np.concatenate([res.results[k]["out"] for k in range(NCORES)], axis=0)
    return outs[meta["node2slot"]].astype(np.float32)

